# revision 14
# baseline (speedup 1.0000x reference)
"""MOELinearDGLFractional Trainium2 kernel.

Data-parallel over systems: 8 cores x 64 systems (512 rows each).

Host prep per core: x is cast to bf16 and pre-transposed to
xt[h, i', c] with column c = 512*s + 128*j + q holding row r = 512*s +
4*q + j (4-row interleave: psum partition q later holds rows 4q..4q+3,
making the output's per-partition HBM chunk 4*256*2B = 2KB contiguous).
Bias is added on the host after gathering (exact fp32), and the bf16
device output is upcast to fp32 on the host.

Per-core pipeline (s = system index, tiles cover 4 systems = 1MB):
  - sync/scalar (HWDGE) DMA xt tiles [128, (h f)] bf16 -> xt_sb,
    deep-buffered (NB_X=10) so the in-stream never stalls on the
    prologue and decouples from PE pacing in the tail
  - PE: per system 16 bf16 matmuls (2 psum banks; per bank 8 matmuls of
    128 cols: {j, j+1} x {moe, reg} x {h0, h1}); moe rhs =
    v3[:, h*64+s, :], reg rhs = resident linw2 tile. Psum holds 4
    systems (8 banks) for a deep pipeline.
  - DVE evacs bank 0, ACT evacs bank 1: psum fp32 -> o_sb bf16.
  - gpsimd (SWDGE) DMA out 1MB per 4 systems (2KB contiguous chunks).

Prologue: V = per-system mixed expert weights [i', hb, o] bf16, computed
on-PE as 32 merged bf16 matmuls (K=128-packed block-diagonal coeffs,
N=8*n_sys, psum-bank rotation) scatter-evacuated to bf16 by DVE/ACT
(GPSIMD cannot read PSUM). PE warm-up runs on c8t (first const in) to
reach the un-throttled 2.4 GHz clock before the prologue.
"""

import sys

sys.path.insert(0, "/opt/trn_rl_repo")

import numpy as np
import ml_dtypes

N_TOTAL = 262144
B = 512
E = 16
I_DIM = 256
O_MOE = 128
O_REG = 128
NCORES = 8
L = 512  # rows per system

NB_X = 10  # xt tile buffers (1MB each, 4 systems)
NB_O = 8  # o_sb buffers (1MB each, 4 systems)


def build_program(n_sys):
    import concourse.bass as bass
    import concourse.mybir as mybir

    f32 = mybir.dt.float32
    bf16 = mybir.dt.bfloat16
    rows = n_sys * L
    hb = 2 * n_sys  # (h, b) combined dim of V
    nldw = 32  # prologue ldw groups (256 chunks / 8)
    pw_n = 8 * n_sys  # prologue psum free size per group
    wn = min(pw_n, 128)  # warmup matmul size
    ntile = n_sys // 4  # 4 systems per in/out transfer

    nc = bass.Bass()
    xt = nc.declare_dram_parameter("xt", [2, 128, rows], bf16, isOutput=False)
    wsb_d = nc.declare_dram_parameter("wsb", [128, 4096], bf16, isOutput=False)
    c8t_d = nc.declare_dram_parameter("c8t", [128, pw_n], bf16, isOutput=False)
    linw_d = nc.declare_dram_parameter("linw", [128, 256], bf16, isOutput=False)
    out = nc.declare_dram_parameter("out", [rows, 256], bf16, isOutput=True)

    xtv = xt.rearrange("h p (k f) -> k p h f", f=2048)
    ov = out.rearrange("(u s4 q j) m -> u q s4 (j m)", s4=4, q=128, j=4)
    ov2 = out.rearrange("(w s2 q j) m -> w q s2 (j m)", s2=2, q=128, j=4)
    ov1 = out.rearrange("(s q j) m -> s q (j m)", q=128, j=4)  # 256KB view

    from contextlib import ExitStack

    with ExitStack() as ctx:
        en = ctx.enter_context
        wsb = en(nc.sbuf_tensor("wsb_sb", [128, 4096], bf16))
        c8t = en(nc.sbuf_tensor("c8t_sb", [128, pw_n], bf16))
        linw2 = en(nc.sbuf_tensor("linw_sb", [128, 256], bf16))
        # V: [i', hb, o] bf16 (moe mixed expert weights, per system)
        v3 = en(nc.sbuf_tensor("v3_sb", [128, hb, 128], bf16))
        # xt tiles: [128, (h f)] per 4-system tile
        xt_sb = [en(nc.sbuf_tensor(f"xt_sb{i}", [128, 4096], bf16)) for i in range(NB_X)]
        o_sb = [en(nc.sbuf_tensor(f"o_sb{i}", [128, 4096], bf16)) for i in range(NB_O)]
        # PSUM: 8 banks exactly; pp[s%4][k] = bank for j-pair k of system s
        pp = [
            [en(nc.psum_tensor(f"pp{i}{k}", [128, 512], f32)) for k in range(2)]
            for i in range(4)
        ]

        sem_names = (
            ["cstC", "cstWa", "cstWb", "cstWc", "cstWd", "cstL", "mm", "dveE",
             "actE", "pw", "pweA", "pweB"]
            + [f"xin{i}" for i in range(NB_X)]
            + [f"dout{i}" for i in range(NB_O)]
        )
        sems = {n: en(nc.semaphore(n)) for n in sem_names}
        mm_s, dveE_s, actE_s, pw_s = (sems[n] for n in ["mm", "dveE", "actE", "pw"])
        pwe_s = [sems[n] for n in ["pweA", "pweB"]]
        xin = [sems[f"xin{i}"] for i in range(NB_X)]
        dout = [sems[f"dout{i}"] for i in range(NB_O)]
        # prologue evac engine assignment: g -> g%2 in (DVE, ACT)
        pwe_total = [len([g for g in range(nldw) if g % 2 == e]) for e in range(2)]

        def wait_pwe(eng, g):
            eng.wait_ge(pwe_s[g % 2], g // 2 + 1)

        def prologue_evac(eng, g, e):
            eng.wait_ge(pw_s, g + 1)
            h = g // 16
            o0 = 8 * (g % 16)
            src = pp[g % 4][0][:, 0:pw_n].rearrange("p (v b) -> p b v", v=8)
            dst = v3[:, h * n_sys : (h + 1) * n_sys, o0 : o0 + 8]
            if e == 0:
                nc.vector.tensor_copy(dst, src).then_inc(pwe_s[0], 1)
            else:
                nc.scalar.copy(out=dst, in_=src).then_inc(pwe_s[1], 1)

        # out-transfer plan: u=0 split per-system (Pool), last tile split
        # into halves; remaining odd u on the SP ring (HWDGE q1, which
        # peaks ~360GB/s vs SWDGE's ~320), even u on Pool (SWDGE)
        tl = ntile - 1
        dout_total = [0] * NB_O
        dout_total[0] += 4 * 16  # u=0 split
        for u in range(1, ntile):
            dout_total[u % NB_O] += 32 if u == tl else 16

        def out_dma(eng, u, part=None):
            if u == tl and tl >= 1:
                # last tile in two halves, one per ring, for a parallel tail
                p2 = part if part is not None else 1
                eng.wait_ge(dveE_s, 4 * tl + 2 * (p2 + 1))
                eng.wait_ge(actE_s, 4 * tl + 2 * (p2 + 1))
                eng.dma_start(
                    out=ov2[2 * tl + p2],
                    in_=o_sb[tl % NB_O][:, 2048 * p2 : 2048 * p2 + 2048],
                ).then_inc(dout[tl % NB_O], 16)
            else:
                eng.wait_ge(dveE_s, 4 * u + 4)
                eng.wait_ge(actE_s, 4 * u + 4)
                eng.dma_start(out=ov[u], in_=o_sb[u % NB_O][:]).then_inc(
                    dout[u % NB_O], 16
                )

        def osb_wait(eng, s):
            u = s // 4
            if u >= NB_O and s % 4 == 0:
                # buffer 0's first tile produced 4 per-system incs (+48),
                # the last tile's split adds one extra inc on its buffer
                extra = 48 if u % NB_O == 0 else 0
                eng.wait_ge(dout[u % NB_O], 16 * (u // NB_O) + extra)

        block = en(nc.Block())

        @block.sync
        def _(sync):
            # wsb split into quarters across both rings so the prologue's
            # first operand chunk lands as early as possible
            sync.dma_start(out=wsb[:, 0:1024], in_=wsb_d[:, 0:1024]).then_inc(
                sems["cstWa"], 16
            )
            sync.dma_start(out=c8t[:], in_=c8t_d[:]).then_inc(sems["cstC"], 16)
            sync.dma_start(out=wsb[:, 1024:2048], in_=wsb_d[:, 1024:2048]).then_inc(
                sems["cstWb"], 16
            )
            sync.dma_start(out=linw2[:], in_=linw_d[:]).then_inc(sems["cstL"], 16)
            # head xt tiles: odd k here (even k go out wait-free on the
            # scalar ring so the two HWDGE rings ramp together)
            for k in range(1, min(NB_X, ntile), 2):
                sync.dma_start(out=xt_sb[k][:], in_=xtv[k]).then_inc(xin[k], 16)
            # gated in-tiles only: this ring is the pipeline-critical input
            # path, so no out transfers may queue ahead of an in-tile
            for k in range(NB_X, ntile):
                sync.wait_ge(mm_s, 4 * (k - NB_X) + 4)
                sync.dma_start(out=xt_sb[k % NB_X][:], in_=xtv[k]).then_inc(
                    xin[k % NB_X], 16
                )
            for b in range(NB_O):
                if dout_total[b]:
                    sync.wait_ge(dout[b], dout_total[b])

        @block.gpsimd
        def _(gpsimd):
            # out DMA (even u; odd u go out on the SP ring): the first
            # tile is split per-system so the out stream starts as soon as
            # system 0 is evacuated
            for s4 in range(4):
                gpsimd.wait_ge(dveE_s, s4 + 1)
                gpsimd.wait_ge(actE_s, s4 + 1)
                gpsimd.dma_start(
                    out=ov1[s4],
                    in_=o_sb[0][:, s4 * 1024 : (s4 + 1) * 1024],
                ).then_inc(dout[0], 16)
            for u in range(2, ntile, 2):
                out_dma(gpsimd, u)
            if tl >= 1:
                out_dma(gpsimd, tl, part=1)
            for b in range(NB_O):
                if dout_total[b]:
                    gpsimd.wait_ge(dout[b], dout_total[b])

        @block.tensor
        def _(tensor):
            # ---- prologue: V (mixed expert weights), bf16 ----
            # (no HAM warm-up: dense dummy matmuls burn the HAM activity
            # budget and drew a half-clock clamp onto the prologue itself;
            # the p-state ramp costs only ~1.5us spread over the first
            # prologue groups)
            tensor.wait_ge(sems["cstC"], 16)
            wsb_gate = {0: "cstWa", nldw // 4: "cstWb", nldw // 2: "cstWc",
                        3 * nldw // 4: "cstWd"}
            for g in range(nldw):
                if g in wsb_gate:
                    tensor.wait_ge(sems[wsb_gate[g]], 16)
                if g >= 4:
                    wait_pwe(tensor, g - 4)
                inst = nc.tensor.matmul(
                    pp[g % 4][0][:, 0:pw_n],
                    wsb[:, g * 128 : (g + 1) * 128],
                    c8t[:, 0:pw_n],
                    start=True,
                    stop=True,
                )
                inst.then_inc(pw_s, 1)

            # ---- main loop ----
            for s in range(n_sys):
                kb = (s // 4) % NB_X
                off = (s % 4) * 512
                tensor.wait_ge(xin[kb], 16 * (s // (4 * NB_X) + 1))
                if s < 4:
                    tensor.wait_ge(sems["cstL"], 16)  # linw2 landed
                    for e in range(2):
                        tensor.wait_ge(pwe_s[e], pwe_total[e])
                else:
                    tensor.wait_ge(dveE_s, s - 3)
                    tensor.wait_ge(actE_s, s - 3)
                # per bank k: one accumulation group; start zeroes the bank,
                # per-element has_written bits handle first-touch-overwrite
                # vs accumulate within the group
                for k in range(2):
                    bank = pp[s % 4][k]
                    first = True
                    for jj in range(2):
                        j = 2 * k + jj
                        for h in range(2):
                            nc.tensor.matmul(
                                bank[:, jj * 256 : jj * 256 + 128],
                                xt_sb[kb][
                                    :, h * 2048 + off + j * 128 : h * 2048 + off + j * 128 + 128
                                ],
                                v3[:, bass.ds(h * n_sys + s, 1), :],
                                start=first,
                                stop=False,
                            )
                            first = False
                        for h in range(2):
                            inst = nc.tensor.matmul(
                                bank[:, jj * 256 + 128 : jj * 256 + 256],
                                xt_sb[kb][
                                    :, h * 2048 + off + j * 128 : h * 2048 + off + j * 128 + 128
                                ],
                                linw2[:, h * 128 : h * 128 + 128],
                                start=False,
                                stop=(jj == 1 and h == 1),
                            )
                inst.then_inc(mm_s, 1)

        @block.scalar
        def _(scalar):
            # wsb quarters 3+4, then even head xt tiles (concurrent w/ sync)
            scalar.dma_start(
                out=wsb[:, 2048:3072], in_=wsb_d[:, 2048:3072]
            ).then_inc(sems["cstWc"], 16)
            scalar.dma_start(
                out=wsb[:, 3072:4096], in_=wsb_d[:, 3072:4096]
            ).then_inc(sems["cstWd"], 16)
            for k in range(0, min(NB_X, ntile), 2):
                scalar.dma_start(out=xt_sb[k][:], in_=xtv[k]).then_inc(xin[k], 16)
            # prologue evac share (odd g)
            for g in range(1, nldw, 2):
                prologue_evac(scalar, g, 1)
            # main: evac psum bank 1 -> o_sb bf16 (cast folded into evac);
            # odd-u out transfers ride this ring (q10 is idle after the
            # head, and ACT's own evac already satisfied the actE wait)
            for s in range(n_sys):
                scalar.wait_ge(mm_s, s + 1)
                osb_wait(scalar, s)
                nc.scalar.copy(
                    out=o_sb[(s // 4) % NB_O][
                        :, (s % 4) * 1024 + 512 : (s % 4) * 1024 + 1024
                    ],
                    in_=pp[s % 4][1][:],
                ).then_inc(actE_s, 1)
                u = s // 4
                if s % 4 == 3 and u % 2 == 1:
                    if u == tl:
                        out_dma(scalar, u, part=0)
                    else:
                        out_dma(scalar, u)

        @block.vector
        def _(vector):
            # prologue evac: psum [p, (v b)] -> V region [p, b(hb), o]
            # group g covers chunks c = 8g+v, all same h: o = 8*(g%16)+v
            for g in range(0, nldw, 2):
                prologue_evac(vector, g, 0)
            # main: evac psum bank 0 -> o_sb bf16
            for s in range(n_sys):
                vector.wait_ge(mm_s, s + 1)
                osb_wait(vector, s)
                nc.vector.tensor_copy(
                    o_sb[(s // 4) % NB_O][:, (s % 4) * 1024 : (s % 4) * 1024 + 512],
                    pp[s % 4][0][:],
                ).then_inc(dveE_s, 1)

    return nc


def _host_inputs(x, coeff, moe_weights, lin_weight, n_sys, core):
    """Build per-core in_map."""
    # wsb: [16v+e, 128g+i'] = W[e, o(c), 128h(c)+i'], c=8g+v, c = h*128+o
    Wr = np.asarray(moe_weights).reshape(E, 128, 2, 128)  # e,o,h,i'
    ch = Wr.transpose(2, 1, 0, 3).reshape(256, E, 128)  # c=(h,o),e,i'
    wsb = np.ascontiguousarray(
        ch.reshape(32, 8, E, 128).transpose(1, 2, 0, 3).reshape(128, 4096)
    ).astype(ml_dtypes.bfloat16)
    b0 = core * n_sys
    cT = np.asarray(coeff)[b0 : b0 + n_sys].T.astype(np.float32)  # [E, n_sys]
    c8t = np.zeros((128, 8 * n_sys), ml_dtypes.bfloat16)
    for v in range(8):
        c8t[16 * v : 16 * v + E, v * n_sys : (v + 1) * n_sys] = cT.astype(
            ml_dtypes.bfloat16
        )
    lw = np.asarray(lin_weight)  # [128, 256]
    linw = np.ascontiguousarray(
        lw.reshape(128, 2, 128).transpose(2, 1, 0).reshape(128, 256)
    ).astype(ml_dtypes.bfloat16)
    rows = n_sys * L
    xs = np.asarray(x)[core * rows : (core + 1) * rows].astype(ml_dtypes.bfloat16)
    # xt[h, i', 512s+128j+q] = x[512s+4q+j, 128h+i']
    xtp = np.ascontiguousarray(
        xs.reshape(n_sys, 128, 4, 2, 128).transpose(3, 4, 0, 2, 1).reshape(2, 128, rows)
    )
    return {"xt": xtp, "wsb": wsb, "c8t": c8t, "linw": linw}


_CACHE = {}


def kernel(
    x,
    expert_mixing_coefficients,
    routing_idxs,
    moe_weights,
    moe_bias,
    lin_weight,
    lin_bias,
    trace=False,
    trace_cores=None,
):
    from concourse.bass_utils import run_bass_kernel_spmd

    n_sys = B // NCORES
    if "nc" not in _CACHE:
        _CACHE["nc"] = build_program(n_sys)
    nc = _CACHE["nc"]
    in_maps = [
        _host_inputs(x, expert_mixing_coefficients, moe_weights, lin_weight, n_sys, c)
        for c in range(NCORES)
    ]
    res = run_bass_kernel_spmd(
        nc, in_maps, list(range(NCORES)), trace=trace, trace_cores=trace_cores
    )
    outs = [res.results[c]["out"] for c in range(NCORES)]
    full = np.concatenate(outs, axis=0).astype(np.float32)
    bias_cat = np.concatenate(
        [np.asarray(moe_bias), np.asarray(lin_bias)]
    ).astype(np.float32)
    full += bias_cat[None, :]
    if trace:
        return full, res
    return full
